# revision 1
# baseline (speedup 1.0000x reference)
"""Self-contained TP-over-heads DeepseekAttention kernel for 8 TRN2 cores.

Sharding: tensor-parallel across heads (4 heads/core). Each core computes
Q/K/V projections for its heads (bf16 matmuls), RoPE, attention with
transposed-scores layout (fp32r matmuls), a row-parallel partial o_proj
fused into the attention loop, then chunked ReduceScatter(add) over the
sequence dim. Host reassembles the 8 per-core [256, 4096] slices.
"""

import numpy as np
import ml_dtypes

import concourse.bass as bass
import concourse.mybir as mybir
import concourse.tile as tile
from concourse import bacc
from concourse.bass_utils import run_bass_kernel_spmd

# problem shapes (hardcoded per contract)
S = 2048
H = 4096
NH = 32
D = 128
NC = 8
HPC = NH // NC          # 4 heads per core
DPC = HPC * D           # 512 head-dims per core
KT = H // 128           # 32 contraction tiles over hidden
SCH = 512               # s-chunk for projections
NSC = S // SCH          # 4
ST = S // 128           # 16 s-tiles
QCH = 512               # q-chunk in attention
NQC = S // QCH          # 4
NKT = S // 128          # 16 k-tiles in attention
SPC = S // NC           # 256 rows of output per core
RS_CHUNKS = 4           # chunked ReduceScatter count (1 or NQC)

f32 = mybir.dt.float32
f32r = mybir.dt.float32r
bf16 = mybir.dt.bfloat16
bf16_np = ml_dtypes.bfloat16

ROPE_THETA = 10000.0
SCALE = float(1.0 / np.sqrt(D))

_CACHE: dict = {}


def _build(with_collective=True):
    from contextlib import ExitStack

    nc = bacc.Bacc("TRN2", target_bir_lowering=False, debug=False, num_devices=NC)

    # ---- I/O ----
    xt = nc.dram_tensor("xt", [KT, 128, S], bf16, kind="ExternalInput").ap()
    wq = nc.dram_tensor("wq", [KT, 128, DPC], bf16, kind="ExternalInput").ap()
    wk = nc.dram_tensor("wk", [KT, 128, DPC], bf16, kind="ExternalInput").ap()
    wv = nc.dram_tensor("wv", [KT, 128, DPC], bf16, kind="ExternalInput").ap()
    wo = nc.dram_tensor("wo", [HPC, 128, H], f32r, kind="ExternalInput").ap()
    cost = nc.dram_tensor("cost", [128, S], f32, kind="ExternalInput").ap()
    sint = nc.dram_tensor("sint", [128, S], f32, kind="ExternalInput").ap()
    rmat = nc.dram_tensor("rmat", [128, 128], f32r, kind="ExternalInput").ap()
    ones_col = nc.dram_tensor("ones_col", [128, 1], f32r, kind="ExternalInput").ap()
    ones_row = nc.dram_tensor("ones_row", [1, 128], f32r, kind="ExternalInput").ap()
    out_ext = nc.dram_tensor("out", [SPC, H], f32, kind="ExternalOutput").ap()

    with tile.TileContext(nc) as tc:
        with (
            tc.tile_pool(name="dram", bufs=1, space="DRAM") as dram_pool,
            tc.tile_pool(name="vstore", bufs=1) as v_store,
        ):
            qkrope = dram_pool.tile([2 * HPC, 128, S], f32r,
                                    name="qkrope")  # [q0..q3, k0..k3]
            partial = dram_pool.tile([S, H], f32, name="partial")
            rs_out = dram_pool.tile([SPC, H], f32, name="rs_out")

            with tc.tile_pool(name="wqk", bufs=1) as wqk_pool:
                wq_sb = wqk_pool.tile([128, KT, DPC], bf16, tag="wq")
                wk_sb = wqk_pool.tile([128, KT, DPC], bf16, tag="wk")

                # ====== Phase V: V projection (natural [s, d] layout) ========
                v_tiles = []
                with (
                    tc.tile_pool(name="wv", bufs=1) as wv_pool,
                    tc.tile_pool(name="xt2", bufs=3) as xt2_pool,
                    tc.tile_pool(name="psB", bufs=2, space="PSUM") as psB,
                ):
                    wv_sb = wv_pool.tile([128, KT, DPC], bf16, tag="wv")
                    for g in range(4):  # split so first matmuls start early
                        nc.sync.dma_start(
                            wv_sb[:, 8 * g:8 * (g + 1), :],
                            wv.rearrange("k p n -> p k n")[:, 8 * g:8 * (g + 1), :],
                        )
                    # prefetch Wq/Wk on the scalar-engine HWDGE queue
                    for g in range(4):
                        nc.scalar.dma_start(
                            wq_sb[:, :, 128 * g:128 * (g + 1)],
                            wq.rearrange("k p n -> p k n")[:, :, 128 * g:128 * (g + 1)],
                        )
                        nc.scalar.dma_start(
                            wk_sb[:, :, 128 * g:128 * (g + 1)],
                            wk.rearrange("k p n -> p k n")[:, :, 128 * g:128 * (g + 1)],
                        )
                    for st in range(ST):
                        x_sb = xt2_pool.tile([128, KT, 128], bf16, tag="x2")
                        nc.sync.dma_start(
                            x_sb[:],
                            xt.rearrange("k p s -> p k s")[:, :,
                                                           st * 128:(st + 1) * 128],
                        )
                        ps = psB.tile([128, DPC], f32, tag="vp")
                        for kt in range(KT):
                            nc.tensor.matmul(
                                ps[:], x_sb[:, kt, :], wv_sb[:, kt, :],
                                start=(kt == 0), stop=(kt == KT - 1),
                            )
                        v_t = v_store.tile([128, DPC], f32r, tag=f"v{st}",
                                           name=f"v{st}")
                        nc.scalar.copy(v_t[:], ps[:])
                        v_tiles.append(v_t)

                # ====== Phase QK: Q/K projections + RoPE (spill to DRAM) =====
                with (
                    tc.tile_pool(name="xt1", bufs=2) as xt1_pool,
                    tc.tile_pool(name="ropec", bufs=2) as rope_pool,
                    tc.tile_pool(name="rmp", bufs=1) as rm_pool,
                    tc.tile_pool(name="qktmp", bufs=2) as qktmp_pool,
                    tc.tile_pool(name="psA", bufs=2, space="PSUM") as psA,
                ):
                    rm_sb = rm_pool.tile([128, 128], f32r, tag="rm")
                    nc.sync.dma_start(rm_sb[:], rmat[:])
                    for sc in range(NSC):
                        s0 = sc * SCH
                        x_sb = xt1_pool.tile([128, KT, SCH], bf16, tag="x")
                        nsplit = 4 if sc == 0 else 1
                        for g in range(nsplit):
                            kspan = KT // nsplit
                            nc.sync.dma_start(
                                x_sb[:, kspan * g:kspan * (g + 1), :],
                                xt.rearrange("k p s -> p k s")[
                                    :, kspan * g:kspan * (g + 1), s0:s0 + SCH],
                            )
                        cos_sb = rope_pool.tile([128, SCH], f32, tag="cos")
                        sin_sb = rope_pool.tile([128, SCH], f32, tag="sin")
                        nc.scalar.dma_start(cos_sb[:], cost[:, s0:s0 + SCH])
                        nc.scalar.dma_start(sin_sb[:], sint[:, s0:s0 + SCH])
                        for pi, w_sb in ((0, wq_sb), (1, wk_sb)):
                            for h in range(HPC):
                                ps = psA.tile([128, SCH], f32, tag="proj")
                                for kt in range(KT):
                                    nc.tensor.matmul(
                                        ps[:],
                                        w_sb[:, kt, h * 128:(h + 1) * 128],
                                        x_sb[:, kt, :],
                                        start=(kt == 0),
                                        stop=(kt == KT - 1),
                                    )
                                raw = qktmp_pool.tile([128, SCH], f32r, tag="raw")
                                nc.scalar.copy(raw[:], ps[:])
                                psr = psA.tile([128, SCH], f32, tag="rot")
                                nc.tensor.matmul(psr[:], rm_sb[:], raw[:],
                                                 start=True, stop=True)
                                t1 = qktmp_pool.tile([128, SCH], f32, tag="t1")
                                nc.vector.tensor_mul(t1[:], raw[:], cos_sb[:])
                                t2 = qktmp_pool.tile([128, SCH], f32, tag="t2")
                                nc.vector.tensor_mul(t2[:], psr[:], sin_sb[:])
                                rope_t = qktmp_pool.tile([128, SCH], f32r,
                                                         tag="rope")
                                nc.vector.tensor_add(rope_t[:], t1[:], t2[:])
                                nc.sync.dma_start(
                                    qkrope[pi * HPC + h, :, s0:s0 + SCH],
                                    rope_t[:],
                                )

            # ====== Phase attn+o_proj: fused, qc-outer ======================
            with (
                tc.tile_pool(name="wo", bufs=1) as wo_pool,
                tc.tile_pool(name="ksb", bufs=2) as k_pool,
                tc.tile_pool(name="qsb", bufs=2) as q_pool,
                tc.tile_pool(name="pt", bufs=8) as pt_pool,
                tc.tile_pool(name="tmp", bufs=1) as tmp_pool,
                tc.tile_pool(name="attnmisc", bufs=3) as misc_pool,
                tc.tile_pool(name="otp", bufs=2) as ot_pool,
                tc.tile_pool(name="drain", bufs=4) as drain_pool,
                tc.tile_pool(name="psC", bufs=1, space="PSUM") as psC,
            ):
                wo_sb = wo_pool.tile([128, HPC, H], f32r, tag="wo")
                for g in range(4):
                    nc.scalar.dma_start(
                        wo_sb[:, g, :],
                        wo.rearrange("h p n -> p h n")[:, g, :],
                    )
                oc_sb = misc_pool.tile([128, 1], f32r, tag="ones_c", bufs=1)
                or_sb = misc_pool.tile([1, 128], f32r, tag="ones_r", bufs=1)
                nc.sync.dma_start(oc_sb[:], ones_col[:])
                nc.sync.dma_start(or_sb[:], ones_row[:])

                for qc in range(NQC):
                    q0 = qc * QCH
                    ot_cur = []
                    for h in range(HPC):
                        k_sb = k_pool.tile([128, S], f32r, tag="k")
                        nc.sync.dma_start(k_sb[:], qkrope[HPC + h])
                        q_sb = q_pool.tile([128, QCH], f32r, tag="q")
                        nc.sync.dma_start(q_sb[:], qkrope[h, :, q0:q0 + QCH])

                        # scores^T + exp, interleaved with attn@V accumulation
                        ps_o = psC.tile([128, QCH], f32, tag="vmm", bufs=2)
                        pts = []
                        for kt in range(NKT):
                            ps_s = psC.tile([128, QCH], f32, tag="scores",
                                            bufs=3)
                            nc.tensor.matmul(
                                ps_s[:],
                                k_sb[:, kt * 128:(kt + 1) * 128],
                                q_sb[:],
                                start=True, stop=True,
                            )
                            pt = pt_pool.tile([128, QCH], f32r, tag="pt")
                            nc.scalar.activation(
                                pt[:], ps_s[:],
                                mybir.ActivationFunctionType.Exp, scale=SCALE,
                            )
                            pts.append(pt)
                            if kt >= 2:
                                kv = kt - 2
                                nc.tensor.matmul(
                                    ps_o[:],
                                    v_tiles[kv][:, h * 128:(h + 1) * 128],
                                    pts[kv][:],
                                    start=(kv == 0), stop=False,
                                )
                        for kv in (NKT - 2, NKT - 1):
                            nc.tensor.matmul(
                                ps_o[:],
                                v_tiles[kv][:, h * 128:(h + 1) * 128],
                                pts[kv][:],
                                start=False, stop=(kv == NKT - 1),
                            )

                        # denominator: batched tree sum of the 16 P^T tiles
                        tmp = tmp_pool.tile([128, 8, QCH], f32, tag="tr")
                        for i in range(8):
                            nc.vector.tensor_add(tmp[:, i, :],
                                                 pts[2 * i][:], pts[2 * i + 1][:])
                        nc.vector.tensor_add(tmp[:, 0:4, :],
                                             tmp[:, 0:4, :], tmp[:, 4:8, :])
                        nc.vector.tensor_add(tmp[:, 0:2, :],
                                             tmp[:, 0:2, :], tmp[:, 2:4, :])
                        t_sum = misc_pool.tile([128, QCH], f32r, tag="tsum",
                                               bufs=2)
                        nc.vector.tensor_add(t_sum[:], tmp[:, 0, :], tmp[:, 1, :])

                        # cross-partition sum -> broadcast -> reciprocal
                        ps_sum = psC.tile([1, QCH], f32, tag="sumbc", bufs=1)
                        nc.tensor.matmul(ps_sum[:], oc_sb[:], t_sum[:],
                                         start=True, stop=True)
                        sum_sb = misc_pool.tile([1, QCH], f32r, tag="sum_sb")
                        nc.vector.tensor_copy(sum_sb[:], ps_sum[:])
                        ps_bc = psC.tile([128, QCH], f32, tag="sumbc", bufs=1)
                        nc.tensor.matmul(ps_bc[:], or_sb[:], sum_sb[:],
                                         start=True, stop=True)
                        recip_sb = misc_pool.tile([128, QCH], f32, tag="recip")
                        nc.vector.reciprocal(recip_sb[:], ps_bc[:])

                        ot_t = ot_pool.tile([128, QCH], f32r, tag=f"ot{h}",
                                            name=f"ot{h}")
                        nc.vector.tensor_mul(ot_t[:], ps_o[:], recip_sb[:])
                        ot_cur.append(ot_t)

                    # fused o_proj for this q-chunk
                    for qt_local in range(QCH // 128):
                        qt = qc * (QCH // 128) + qt_local
                        for nci in range(H // 512):
                            n0 = nci * 512
                            ps = psC.tile([128, 512], f32, tag="op", bufs=2)
                            for h in range(HPC):
                                nc.tensor.matmul(
                                    ps[:],
                                    ot_cur[h][:,
                                              qt_local * 128:(qt_local + 1) * 128],
                                    wo_sb[:, h, n0:n0 + 512],
                                    start=(h == 0), stop=(h == HPC - 1),
                                )
                            dr = drain_pool.tile([128, 512], f32, tag="dr")
                            nc.vector.tensor_copy(dr[:], ps[:])
                            nc.gpsimd.dma_start(
                                partial[qt * 128:(qt + 1) * 128, n0:n0 + 512],
                                dr[:],
                            )

                    # chunked ReduceScatter over this q-chunk's rows
                    if with_collective and RS_CHUNKS == NQC:
                        nc.gpsimd.collective_compute(
                            "ReduceScatter",
                            mybir.AluOpType.add,
                            replica_groups=[list(range(NC))],
                            ins=[partial[q0:q0 + QCH, :].opt()],
                            outs=[rs_out[qc * (QCH // NC):
                                         (qc + 1) * (QCH // NC), :].opt()],
                        )

            # ====== Final: (single RS) + output ==============================
            if with_collective and RS_CHUNKS != NQC:
                nc.gpsimd.collective_compute(
                    "ReduceScatter",
                    mybir.AluOpType.add,
                    replica_groups=[list(range(NC))],
                    ins=[partial.opt()],
                    outs=[rs_out.opt()],
                )
            if with_collective:
                nc.gpsimd.dma_start(out_ext[:], rs_out[:])
            else:
                nc.gpsimd.dma_start(out_ext[:], partial[:SPC, :])

    nc.compile()
    return nc


def _host_prep(positions, hidden_states, Wq, Wk, Wv, Wo):
    X = np.asarray(hidden_states, dtype=np.float32).reshape(S, H)
    XT = np.ascontiguousarray(X.T).astype(bf16_np).reshape(KT, 128, S)

    pos = np.asarray(positions).astype(np.float32)
    inv_freq = (1.0 / (ROPE_THETA ** (np.arange(0, D, 2, dtype=np.float32) / D)))
    freqs = pos[:, None] * inv_freq[None, :]
    emb = np.concatenate([freqs, freqs], axis=-1)        # [S, D]
    cosT = np.ascontiguousarray(np.cos(emb).astype(np.float32).T)  # [128, S]
    sinT = np.ascontiguousarray(np.sin(emb).astype(np.float32).T)

    rm = np.zeros((128, 128), np.float32)
    idx = np.arange(64)
    rm[64 + idx, idx] = -1.0   # out[0:64]  = -in[64:128]
    rm[idx, 64 + idx] = 1.0    # out[64:128] = in[0:64]

    Wq = np.asarray(Wq, dtype=np.float32)
    Wk = np.asarray(Wk, dtype=np.float32)
    Wv = np.asarray(Wv, dtype=np.float32)
    Wo = np.asarray(Wo, dtype=np.float32)

    in_maps = []
    for c in range(NC):
        sl = slice(DPC * c, DPC * (c + 1))
        wq_c = np.ascontiguousarray(Wq[sl, :].T).astype(bf16_np).reshape(KT, 128, DPC)
        wk_c = np.ascontiguousarray(Wk[sl, :].T).astype(bf16_np).reshape(KT, 128, DPC)
        wv_c = np.ascontiguousarray(Wv[sl, :].T).astype(bf16_np).reshape(KT, 128, DPC)
        wo_c = np.ascontiguousarray(Wo[:, sl].T).reshape(HPC, 128, H)
        in_maps.append({
            "xt": XT, "wq": wq_c, "wk": wk_c, "wv": wv_c, "wo": wo_c,
            "cost": cosT, "sint": sinT, "rmat": rm,
            "ones_col": np.ones((128, 1), np.float32),
            "ones_row": np.ones((1, 128), np.float32),
        })
    return in_maps


def _assemble(results):
    """Reassemble full [1, S, H] output from per-core RS slices."""
    if RS_CHUNKS == NQC:
        # core c, chunk qc holds global rows qc*QCH + c*(QCH//NC) + r
        full = np.empty((NQC, NC, QCH // NC, H), np.float32)
        for c in range(NC):
            full[:, c] = results[c]["out"].reshape(NQC, QCH // NC, H)
        return full.reshape(1, S, H)
    out = np.concatenate([results[c]["out"] for c in range(NC)], axis=0)
    return out.reshape(1, S, H)


def kernel(positions, hidden_states, Wq, Wk, Wv, Wo):
    if "nc" not in _CACHE:
        _CACHE["nc"] = _build()
    nc = _CACHE["nc"]
    in_maps = _host_prep(positions, hidden_states, Wq, Wk, Wv, Wo)
    res = run_bass_kernel_spmd(nc, in_maps, list(range(NC)))
    return _assemble(res.results).astype(np.float32)



# revision 8
# speedup vs baseline: 3.3855x; 3.3855x over previous
"""Self-contained TP-over-heads DeepseekAttention kernel for 8 TRN2 cores.

v2: SBUF-resident Q/K/V (no DRAM spill), weight-streaming passes (Q, K, V)
over x, bf16 probabilities/V/Wo/partials, Pool-engine drains, chunked bf16
ReduceScatter. Each core handles 4 heads end-to-end; host reassembles the
8 per-core [256, 4096] row slices.
"""

import numpy as np
import ml_dtypes

import concourse.bass as bass
import concourse.mybir as mybir
import concourse.tile as tile
from concourse import bacc
from concourse.bass_utils import run_bass_kernel_spmd

# problem shapes (hardcoded per contract)
S = 2048
H = 4096
NH = 32
D = 128
NC = 8
HPC = NH // NC          # 4 heads per core
DPC = HPC * D           # 512 head-dims per core
KT = H // 128           # 32 contraction tiles over hidden
KTH = KT // 2           # kt half
SCH = 512               # s-chunk for projections
NSC = S // SCH          # 4
ST = S // 128           # 16 s-tiles
QCH = 512               # q-chunk in attention
NQC = S // QCH          # 4
NKT = S // 128          # 16 k-tiles in attention
SPC = S // NC           # 256 rows of output per core

f32 = mybir.dt.float32
f32r = mybir.dt.float32r
bf16 = mybir.dt.bfloat16
bf16_np = ml_dtypes.bfloat16

ROPE_THETA = 10000.0
SCALE = float(1.0 / np.sqrt(D))

_CACHE: dict = {}


def _build(with_collective=True, pdt_bf16=True):
    nc = bacc.Bacc("TRN2", target_bir_lowering=False, debug=False, num_devices=NC)

    # ---- I/O ----
    xt = nc.dram_tensor("xt", [KT, 128, S], bf16, kind="ExternalInput").ap()
    wq = nc.dram_tensor("wq", [KT, 128, DPC], bf16, kind="ExternalInput").ap()
    wk = nc.dram_tensor("wk", [KT, 128, DPC], bf16, kind="ExternalInput").ap()
    wv = nc.dram_tensor("wv", [KT, 128, DPC], bf16, kind="ExternalInput").ap()
    wo = nc.dram_tensor("wo", [HPC, 128, H], bf16, kind="ExternalInput").ap()
    cost = nc.dram_tensor("cost", [128, S], f32, kind="ExternalInput").ap()
    sint = nc.dram_tensor("sint", [128, S], f32, kind="ExternalInput").ap()
    rmat = nc.dram_tensor("rmat", [128, 128], f32r, kind="ExternalInput").ap()
    ones_col = nc.dram_tensor("ones_col", [128, 1], f32r, kind="ExternalInput").ap()
    ones_row = nc.dram_tensor("ones_row", [1, 128], f32r, kind="ExternalInput").ap()
    out_ext = nc.dram_tensor("out", [SPC, H], f32, kind="ExternalOutput").ap()

    PDT = bf16 if pdt_bf16 else f32
    xt_p = xt.rearrange("k p s -> p k s")

    with tile.TileContext(nc) as tc:
        with (
            tc.tile_pool(name="dram", bufs=1, space="DRAM") as dram_pool,
            tc.tile_pool(name="store", bufs=1) as store,
        ):
            partial = dram_pool.tile([S, H], PDT, name="partial")
            rs_out = dram_pool.tile([SPC, H], PDT, name="rs_out")

            q_st = store.tile([128, HPC, S], f32r, tag="q_st")
            k_st = store.tile([128, HPC, S], f32r, tag="k_st")
            v_st = store.tile([128, ST, DPC], bf16, tag="v_st")
            rm_sb = store.tile([128, 128], f32r, tag="rm")
            oc_sb = store.tile([128, 1], f32r, tag="oc")
            or_sb = store.tile([1, 128], f32r, tag="or")
            nc.scalar.dma_start(rm_sb[:], rmat[:])
            nc.scalar.dma_start(oc_sb[:], ones_col[:])
            nc.scalar.dma_start(or_sb[:], ones_row[:])

            # ====== projection passes: Q, K then V (weights streamed) =======
            with (
                tc.tile_pool(name="wp", bufs=3) as wpool,
                tc.tile_pool(name="xp", bufs=3) as xpool,
                tc.tile_pool(name="csp", bufs=2) as cspool,
                tc.tile_pool(name="rawp", bufs=3) as rawpool,
                tc.tile_pool(name="ttp", bufs=2) as ttpool,
                tc.tile_pool(name="psA", bufs=4, space="PSUM") as psA,
                tc.tile_pool(name="psR", bufs=2, space="PSUM") as psR,
            ):
                # weight halves stream through a 3-deep ring; allocations are
                # emitted per pass so ring-reuse WAR deps see all readers.
                def load_w_half(wsrc, hf):
                    wt = wpool.tile([128, KTH, DPC], bf16, tag="w")
                    nc.sync.dma_start(
                        wt[:],
                        wsrc.rearrange("k p n -> p k n")[
                            :, KTH * hf:KTH * (hf + 1), :],
                    )
                    return wt

                # prefetch plan: [Q: wq0, wq1, wk0] [K: wk1, wv0] [V: wv1]
                pending = [(wq, 0), (wq, 1), (wk, 0), (wk, 1),
                           (wv, 0), (wv, 1)]
                loaded = [load_w_half(*pending[i]) for i in range(3)]
                next_load = 3

                # --- Q and K passes: out [128 odim, s] + RoPE -> q_st/k_st
                for pi, dst in ((0, q_st), (1, k_st)):
                    wlo, whi = loaded[2 * pi], loaded[2 * pi + 1]
                    for sc in range(NSC):
                        s0 = sc * SCH
                        cos_sb = cspool.tile([128, SCH], f32, tag="cos")
                        sin_sb = cspool.tile([128, SCH], f32, tag="sin")
                        nc.scalar.dma_start(cos_sb[:], cost[:, s0:s0 + SCH])
                        nc.scalar.dma_start(sin_sb[:], sint[:, s0:s0 + SCH])
                        x_lo = xpool.tile([128, KTH, SCH], bf16, tag="x")
                        nc.gpsimd.dma_start(x_lo[:], xt_p[:, 0:KTH, s0:s0 + SCH])
                        x_hi = xpool.tile([128, KTH, SCH], bf16, tag="x")
                        nc.gpsimd.dma_start(x_hi[:], xt_p[:, KTH:KT, s0:s0 + SCH])

                        pss = [psA.tile([128, SCH], f32, tag="proj", name=f"pj{_h}")
                               for _h in range(HPC)]
                        for wt, x_sb, base in ((wlo, x_lo, 0), (whi, x_hi, KTH)):
                            for kt in range(KTH):
                                for h in range(HPC):
                                    nc.tensor.matmul(
                                        pss[h][:],
                                        wt[:, kt, h * 128:(h + 1) * 128],
                                        x_sb[:, kt, :],
                                        start=(base + kt == 0),
                                        stop=(base + kt == KT - 1),
                                    )
                        for h in range(HPC):
                            raw = rawpool.tile([128, SCH], f32r, tag="raw")
                            nc.scalar.copy(raw[:], pss[h][:])
                            psr = psR.tile([128, SCH], f32, tag="rot")
                            nc.tensor.matmul(psr[:], rm_sb[:], raw[:],
                                             start=True, stop=True)
                            t1 = ttpool.tile([128, SCH], f32, tag="t1")
                            nc.vector.tensor_mul(t1[:], raw[:], cos_sb[:])
                            t2 = ttpool.tile([128, SCH], f32, tag="t2")
                            nc.vector.tensor_mul(t2[:], psr[:], sin_sb[:])
                            nc.vector.tensor_add(
                                dst[:, h, s0:s0 + SCH], t1[:], t2[:])

                    # emit next pass's weight loads now that this pass's
                    # readers exist (ring WAR deps double as prefetch)
                    n_pref = 2 if pi == 0 else 1
                    for _ in range(n_pref):
                        loaded.append(load_w_half(*pending[next_load]))
                        next_load += 1

                # --- V pass: natural [s, d] layout -> v_st (bf16)
                wlo, whi = loaded[4], loaded[5]
                with tc.tile_pool(name="psV", bufs=2, space="PSUM") as psV:
                    for sc in range(NSC):
                        s0 = sc * SCH
                        x_lo = xpool.tile([128, KTH, SCH], bf16, tag="x")
                        nc.gpsimd.dma_start(x_lo[:], xt_p[:, 0:KTH, s0:s0 + SCH])
                        x_hi = xpool.tile([128, KTH, SCH], bf16, tag="x")
                        nc.gpsimd.dma_start(x_hi[:], xt_p[:, KTH:KT, s0:s0 + SCH])
                        for stl in range(SCH // 128):
                            st = sc * (SCH // 128) + stl
                            ps = psV.tile([128, DPC], f32, tag="vp")
                            for wt, x_sb, base in ((wlo, x_lo, 0),
                                                   (whi, x_hi, KTH)):
                                for kt in range(KTH):
                                    nc.tensor.matmul(
                                        ps[:],
                                        x_sb[:, kt, stl * 128:(stl + 1) * 128],
                                        wt[:, kt, :],
                                        start=(base + kt == 0),
                                        stop=(base + kt == KT - 1),
                                    )
                            nc.scalar.copy(v_st[:, st, :], ps[:])

            # ====== attention + fused o_proj, qc-outer ======================
            with (
                tc.tile_pool(name="wo", bufs=1) as wo_pool,
                tc.tile_pool(name="pt", bufs=20) as pt_pool,
                tc.tile_pool(name="tmp", bufs=1) as tmp_pool,
                tc.tile_pool(name="attnmisc", bufs=3) as misc_pool,
                tc.tile_pool(name="otp", bufs=8) as ot_pool,
                tc.tile_pool(name="drain", bufs=4) as drain_pool,
                tc.tile_pool(name="psC", bufs=1, space="PSUM") as psC,
            ):
                wo_sb = wo_pool.tile([128, HPC, H], bf16, tag="wo")
                for g in range(4):
                    nc.scalar.dma_start(
                        wo_sb[:, :, 1024 * g:1024 * (g + 1)],
                        wo.rearrange("h p n -> p h n")[:, :,
                                                       1024 * g:1024 * (g + 1)],
                    )

                def emit_oproj_qt(qc_o, qt_local, ots):
                    """One row-tile of o_proj for chunk qc_o: 8 n-chunks x 4
                    heads; drains on Pool, partial writes on the sync queue."""
                    qt = qc_o * (QCH // 128) + qt_local
                    for nci in range(H // 512):
                        n0 = nci * 512
                        ps = psC.tile([128, 512], f32, tag="opx", bufs=2,
                                      name="psop")
                        for h in range(HPC):
                            nc.tensor.matmul(
                                ps[:],
                                ots[h][:, qt_local * 128:(qt_local + 1) * 128],
                                wo_sb[:, h, n0:n0 + 512],
                                start=(h == 0), stop=(h == HPC - 1),
                            )
                        dr = drain_pool.tile([128, 512], PDT, tag="dr")
                        eng = nc.vector if nci % 2 == 0 else nc.scalar
                        if nci % 2 == 0:
                            nc.vector.tensor_copy(dr[:], ps[:])
                        else:
                            nc.scalar.copy(dr[:], ps[:])
                        nc.sync.dma_start(
                            partial[qt * 128:(qt + 1) * 128, n0:n0 + 512],
                            dr[:],
                        )

                def emit_rs(qc_o):
                    nc.gpsimd.collective_compute(
                        "ReduceScatter",
                        mybir.AluOpType.add,
                        replica_groups=[list(range(NC))],
                        ins=[partial[qc_o * QCH:(qc_o + 1) * QCH, :].opt()],
                        outs=[rs_out[qc_o * (QCH // NC):
                                     (qc_o + 1) * (QCH // NC), :].opt()],
                    )

                ot_prev = None
                for qc in range(NQC):
                    q0 = qc * QCH
                    ot_cur = []
                    for h in range(HPC):
                        # scores^T in 2-bank pairs + one exp per pair,
                        # interleaved with attn@V accumulation (lag one pair)
                        ps_o = psC.tile([128, QCH], f32, tag="vmm", bufs=2)
                        pts = []
                        for g in range(NKT // 2):
                            ps_s = psC.tile([128, 2 * QCH], f32, tag="scores",
                                            bufs=2)
                            for j in range(2):
                                kt = 2 * g + j
                                nc.tensor.matmul(
                                    ps_s[:, j * QCH:(j + 1) * QCH],
                                    k_st[:, h, kt * 128:(kt + 1) * 128],
                                    q_st[:, h, q0:q0 + QCH],
                                    start=True, stop=True,
                                )
                            pt = pt_pool.tile([128, 2 * QCH], bf16, tag="pt")
                            nc.scalar.activation(
                                pt[:], ps_s[:],
                                mybir.ActivationFunctionType.Exp, scale=SCALE,
                            )
                            pts.append(pt)
                            if g >= 1:
                                for j in range(2):
                                    kv = 2 * (g - 1) + j
                                    nc.tensor.matmul(
                                        ps_o[:],
                                        v_st[:, kv, h * 128:(h + 1) * 128],
                                        pts[g - 1][:, j * QCH:(j + 1) * QCH],
                                        start=(kv == 0), stop=False,
                                    )
                        for j in range(2):
                            kv = NKT - 2 + j
                            nc.tensor.matmul(
                                ps_o[:],
                                v_st[:, kv, h * 128:(h + 1) * 128],
                                pts[NKT // 2 - 1][:, j * QCH:(j + 1) * QCH],
                                start=False, stop=(kv == NKT - 1),
                            )

                        # denominator: batched tree sum of the 8 P^T pairs
                        tmp = tmp_pool.tile([128, 4, 2 * QCH], f32, tag="tr")
                        for i in range(4):
                            nc.vector.tensor_add(tmp[:, i, :],
                                                 pts[2 * i][:], pts[2 * i + 1][:])
                        nc.vector.tensor_add(tmp[:, 0:2, :],
                                             tmp[:, 0:2, :], tmp[:, 2:4, :])
                        nc.vector.tensor_add(tmp[:, 0, :],
                                             tmp[:, 0, :], tmp[:, 1, :])
                        t_sum = misc_pool.tile([128, QCH], f32r, tag="tsum",
                                               bufs=2)
                        nc.vector.tensor_add(t_sum[:], tmp[:, 0, 0:QCH],
                                             tmp[:, 0, QCH:2 * QCH])

                        # cross-partition sum -> broadcast -> reciprocal
                        ps_sum = psC.tile([1, QCH], f32, tag="opx", bufs=2,
                                          name="pssum")
                        nc.tensor.matmul(ps_sum[:], oc_sb[:], t_sum[:],
                                         start=True, stop=True)
                        sum_sb = misc_pool.tile([1, QCH], f32r, tag="sum_sb")
                        nc.vector.tensor_copy(sum_sb[:], ps_sum[:])
                        ps_bc = psC.tile([128, QCH], f32, tag="opx", bufs=2,
                                         name="psbc")
                        nc.tensor.matmul(ps_bc[:], or_sb[:], sum_sb[:],
                                         start=True, stop=True)
                        recip_sb = misc_pool.tile([128, QCH], f32, tag="recip")
                        nc.vector.reciprocal(recip_sb[:], ps_bc[:])

                        ot_t = ot_pool.tile([128, QCH], bf16, tag="ot")
                        nc.vector.tensor_mul(ot_t[:], ps_o[:], recip_sb[:])
                        ot_cur.append(ot_t)

                        # fill PE stalls with previous chunk's o_proj row-tile
                        if ot_prev is not None:
                            emit_oproj_qt(qc - 1, h, ot_prev)

                    if ot_prev is not None and with_collective:
                        emit_rs(qc - 1)
                    ot_prev = ot_cur

                # last chunk's o_proj + ReduceScatter
                for h in range(HPC):
                    emit_oproj_qt(NQC - 1, h, ot_prev)
                if with_collective:
                    emit_rs(NQC - 1)

            # ====== tail: convert to f32 and store ==========================
            src = rs_out if with_collective else partial
            if pdt_bf16:
                with tc.tile_pool(name="cvt", bufs=2) as cvt_pool:
                    for i in range(SPC // 128):
                        ld = cvt_pool.tile([128, H], bf16, tag="cvt_ld")
                        nc.sync.dma_start(ld[:], src[i * 128:(i + 1) * 128, :])
                        cv = cvt_pool.tile([128, H], f32, tag="cvt_f32")
                        nc.vector.tensor_copy(cv[:], ld[:])
                        nc.gpsimd.dma_start(out_ext[i * 128:(i + 1) * 128, :],
                                            cv[:])
            else:
                nc.gpsimd.dma_start(out_ext[:], src[:SPC, :])

    nc.compile()
    return nc


def _host_prep(positions, hidden_states, Wq, Wk, Wv, Wo):
    X = np.asarray(hidden_states, dtype=np.float32).reshape(S, H)
    XT = np.ascontiguousarray(X.T).astype(bf16_np).reshape(KT, 128, S)

    pos = np.asarray(positions).astype(np.float32)
    inv_freq = (1.0 / (ROPE_THETA ** (np.arange(0, D, 2, dtype=np.float32) / D)))
    freqs = pos[:, None] * inv_freq[None, :]
    emb = np.concatenate([freqs, freqs], axis=-1)        # [S, D]
    cosT = np.ascontiguousarray(np.cos(emb).astype(np.float32).T)  # [128, S]
    sinT = np.ascontiguousarray(np.sin(emb).astype(np.float32).T)

    rm = np.zeros((128, 128), np.float32)
    idx = np.arange(64)
    rm[64 + idx, idx] = -1.0   # out[0:64]  = -in[64:128]
    rm[idx, 64 + idx] = 1.0    # out[64:128] = in[0:64]

    Wq = np.asarray(Wq, dtype=np.float32)
    Wk = np.asarray(Wk, dtype=np.float32)
    Wv = np.asarray(Wv, dtype=np.float32)
    Wo = np.asarray(Wo, dtype=np.float32)

    in_maps = []
    for c in range(NC):
        sl = slice(DPC * c, DPC * (c + 1))
        wq_c = np.ascontiguousarray(Wq[sl, :].T).astype(bf16_np).reshape(KT, 128, DPC)
        wk_c = np.ascontiguousarray(Wk[sl, :].T).astype(bf16_np).reshape(KT, 128, DPC)
        wv_c = np.ascontiguousarray(Wv[sl, :].T).astype(bf16_np).reshape(KT, 128, DPC)
        wo_c = np.ascontiguousarray(Wo[:, sl].T).astype(bf16_np).reshape(HPC, 128, H)
        in_maps.append({
            "xt": XT, "wq": wq_c, "wk": wk_c, "wv": wv_c, "wo": wo_c,
            "cost": cosT, "sint": sinT, "rmat": rm,
            "ones_col": np.ones((128, 1), np.float32),
            "ones_row": np.ones((1, 128), np.float32),
        })
    return in_maps


def _assemble(results):
    """Reassemble full [1, S, H] output from per-core RS slices."""
    # core c, chunk qc holds global rows qc*QCH + c*(QCH//NC) + r
    full = np.empty((NQC, NC, QCH // NC, H), np.float32)
    for c in range(NC):
        full[:, c] = results[c]["out"].reshape(NQC, QCH // NC, H)
    return full.reshape(1, S, H)


def kernel(positions, hidden_states, Wq, Wk, Wv, Wo):
    if "nc" not in _CACHE:
        _CACHE["nc"] = _build()
    nc = _CACHE["nc"]
    in_maps = _host_prep(positions, hidden_states, Wq, Wk, Wv, Wo)
    res = run_bass_kernel_spmd(nc, in_maps, list(range(NC)))
    return _assemble(res.results).astype(np.float32)
